# revision 1
# baseline (speedup 1.0000x reference)
"""MiniGPT forward pass on 8 Trainium2 NeuronCores.

Sharding: sequence-parallel. Core c handles batch g = c//4, token chunk
r = c%4 (512 tokens). All per-token ops (LN, QKV, Wo, FFN, LM head) are
local; K/V are exchanged with two 4-core AllGathers per layer (split by
head halves so the second overlaps first-half attention).

Activations are kept transposed [D, tokens] so every major matmul has
free dim N=512 with contraction on the partition axis. The K/Q/V/
attention-weights path runs in bf16 (fp32 PSUM accumulation); LN,
projections, FFN and LM head run in fp32r. Softmax skips
max-subtraction (pre-softmax scores are O(1) here); masked positions
are zeroed exactly by multiplying exp(s) with a 0/1 mask, and the
denominator comes from a ones-column appended to V inside the same
accumulation matmul.
"""
import sys
sys.path.insert(0, '/opt/trn_rl_repo')
import numpy as np
import concourse.bass as bass
import concourse.bacc as bacc
import concourse.tile as tile
import concourse.mybir as mybir
from concourse.bass_utils import run_bass_kernel_spmd

F32 = mybir.dt.float32
F32R = mybir.dt.float32r
BF16 = mybir.dt.bfloat16
AF = mybir.ActivationFunctionType
ALU = mybir.AluOpType

# model dims
B, S, D, H, DK, F, V = 2, 2048, 1024, 16, 64, 4096, 32000
L = 6
EPS = 1e-5
T = 512           # tokens per core
NT = T // 128     # 4 token tiles
ND = D // 128     # 8 d tiles
NF = F // 128     # 32 f tiles
NK = S // 128     # 16 key tiles
NV = V // 128     # 250 vocab tiles
HPAIRS = H // 2   # 8 head-pair tiles
N_CORES = 8

# one AG half: K rows [h*512:(h+1)*512] (4 hp tiles) + V cols [h*520:(h+1)*520]
KTH_FLAT = 4 * 128 * T          # 262144
VH_FLAT = T * 520               # 266240
KVH_FLAT = KTH_FLAT + VH_FLAT   # 528384
VW = H * 65                     # 1040

_CACHE = {}


def _build(n_layers=L, n_vtiles=NV):
    nc = bacc.Bacc("TRN2", target_bir_lowering=False, debug=False,
                   num_devices=N_CORES)

    def din(name, shape, dt=F32R):
        return nc.dram_tensor(name, shape, dt, kind="ExternalInput").ap()

    x0t = din("x0t", [D, T])
    maskt = din("maskt", [S, T], BF16)
    wq = din("wq", [L, D, D])
    wk = din("wk", [L, D, D])
    wv = din("wv", [L, D, D])
    wo = din("wo", [L, D, D])
    w1s = din("w1s", [L, NF, 128, D])
    w2 = din("w2", [L, F, D])
    woutr = din("woutr", [NV, 128, D])
    ln1g = din("ln1g", [L, D], F32)
    ln1b = din("ln1b", [L, D], F32)
    ln2g = din("ln2g", [L, D], F32)
    ln2b = din("ln2b", [L, D], F32)
    b1 = din("b1", [L, F], F32)
    b2 = din("b2", [L, D], F32)
    lnfg = din("lnfg", [D], F32)
    lnfb = din("lnfb", [D], F32)
    ones_in = din("ones_in", [128])
    onescol = din("onescol", [128, H], BF16)

    logits_t = nc.dram_tensor("logits_t", [n_vtiles * 128, T], F32R,
                              kind="ExternalOutput").ap()

    groups = [[0, 1, 2, 3], [4, 5, 6, 7]]

    with tile.TileContext(nc) as tc:
        with tc.tile_pool(name="sb", bufs=1) as sb, \
             tc.tile_pool(name="ps", bufs=1, space="PSUM") as ps, \
             tc.tile_pool(name="dram", bufs=1, space="DRAM") as dram:

            ones128 = sb.tile([128, 1], F32R, tag="ones128")
            ones1 = sb.tile([1, 128], F32R, tag="ones1")
            nc.sync.dma_start(ones128[:], ones_in[:, None])
            nc.sync.dma_start(ones1[:], ones_in[None, :])

            # persistent residual stream xT: 8 tiles [128, 512]
            xts = []
            for j in range(ND):
                t = sb.tile([128, T], F32R, tag="xt", bufs=ND, name=f"xt{j}")
                nc.sync.dma_start(t[:], x0t[j * 128:(j + 1) * 128, :])
                xts.append(t)

            # causal mask, resident whole kernel
            mts = []
            for k in range(NK):
                t = sb.tile([128, T], BF16, tag="mask", bufs=NK, name=f"mask{k}")
                nc.sync.dma_start(t[:], maskt[k * 128:(k + 1) * 128, :])
                mts.append(t)

            def layer_norm(x_tiles, gcol_t, bcol_t, sfx):
                statx = ps.tile([1, T], F32, tag="stat", bufs=2,
                                name=f"stx{sfx}")
                statq = ps.tile([1, T], F32, tag="stat", bufs=2,
                                name=f"stq{sfx}")
                for j in range(ND):
                    sq = sb.tile([128, T], F32R, tag="work512", bufs=4,
                                 name=f"sq{sfx}")
                    nc.scalar.activation(sq[:], x_tiles[j][:], AF.Square)
                    nc.tensor.matmul(statx[0:1, :], ones128[:], x_tiles[j][:],
                                     start=(j == 0), stop=(j == ND - 1))
                    nc.tensor.matmul(statq[0:1, :], ones128[:], sq[:],
                                     start=(j == 0), stop=(j == ND - 1))
                mean = sb.tile([1, T], F32R, tag="lnsmall", bufs=3,
                               name=f"mean{sfx}")
                nc.vector.tensor_scalar_mul(mean[:], statx[0:1, :], 1.0 / D)
                ex2 = sb.tile([1, T], F32, tag="lnsmall", bufs=3,
                              name=f"ex2{sfx}")
                nc.vector.tensor_scalar_mul(ex2[:], statq[0:1, :], 1.0 / D)
                m2 = sb.tile([1, T], F32, tag="lnsmall", bufs=3,
                             name=f"m2{sfx}")
                nc.scalar.activation(m2[:], mean[:], AF.Square)
                var = sb.tile([1, T], F32, tag="lnsmall", bufs=3,
                              name=f"var{sfx}")
                nc.vector.tensor_sub(var[:], ex2[:], m2[:])
                nc.vector.tensor_scalar_add(var[:], var[:], EPS)
                sd = sb.tile([1, T], F32, tag="lnsmall", bufs=3,
                             name=f"sd{sfx}")
                nc.scalar.activation(sd[:], var[:], AF.Sqrt)
                rstd = sb.tile([1, T], F32R, tag="lnsmall", bufs=3,
                               name=f"rstd{sfx}")
                with nc.allow_low_precision(reason="fp32r matmul feed"):
                    nc.vector.reciprocal(rstd[:], sd[:])
                mb = ps.tile([128, T], F32, tag="sT", bufs=2, name=f"mb{sfx}")
                nc.tensor.matmul(mb[:], ones1[:], mean[:], start=True,
                                 stop=True)
                rb = ps.tile([128, T], F32, tag="sT", bufs=2, name=f"rb{sfx}")
                nc.tensor.matmul(rb[:], ones1[:], rstd[:], start=True,
                                 stop=True)
                h_tiles = []
                for j in range(ND):
                    ht = sb.tile([128, T], F32R, tag="hx", bufs=ND,
                                 name=f"h{sfx}_{j}")
                    nc.vector.tensor_sub(ht[:], x_tiles[j][:], mb[:])
                    nc.vector.tensor_mul(ht[:], ht[:], rb[:])
                    nc.vector.tensor_scalar(ht[:], ht[:], gcol_t[:, j:j + 1],
                                            bcol_t[:, j:j + 1], ALU.mult,
                                            ALU.add)
                    h_tiles.append(ht)
                return h_tiles

            for l in range(n_layers):
                # --- per-layer params ---
                lg1 = sb.tile([128, ND], F32, tag="lnp", bufs=8, name="lg1")
                nc.sync.dma_start(lg1[:], ln1g[l].rearrange("(c p) -> p c", p=128))
                lb1 = sb.tile([128, ND], F32, tag="lnp", bufs=8, name="lb1")
                nc.sync.dma_start(lb1[:], ln1b[l].rearrange("(c p) -> p c", p=128))
                lg2 = sb.tile([128, ND], F32, tag="lnp", bufs=8, name="lg2")
                nc.sync.dma_start(lg2[:], ln2g[l].rearrange("(c p) -> p c", p=128))
                lb2 = sb.tile([128, ND], F32, tag="lnp", bufs=8, name="lb2")
                nc.sync.dma_start(lb2[:], ln2b[l].rearrange("(c p) -> p c", p=128))
                b1t = sb.tile([128, NF], F32, tag="b1t", bufs=2, name="b1t")
                nc.sync.dma_start(b1t[:], b1[l].rearrange("(c p) -> p c", p=128))
                b2t = sb.tile([128, ND], F32, tag="lnp", bufs=8, name="b2t")
                nc.sync.dma_start(b2t[:], b2[l].rearrange("(c p) -> p c", p=128))

                h1 = layer_norm(xts, lg1, lb1, f"a{l}")

                kv_in = [dram.tile([KVH_FLAT], BF16, tag=f"kvin{h}", bufs=2,
                                   name=f"kvin{h}") for h in range(2)]

                # --- K projection (all 8 hp tiles), DMA into both halves ---
                wk_t = []
                for ci in range(ND):
                    t = sb.tile([128, D], F32R, tag="w", bufs=9, name=f"wk{ci}")
                    nc.sync.dma_start(t[:], wk[l][ci * 128:(ci + 1) * 128, :])
                    wk_t.append(t)
                for j in range(ND):
                    mm = ps.tile([128, T], F32, tag="mm", bufs=2, name="kmm")
                    for ci in range(ND):
                        nc.tensor.matmul(mm[:], wk_t[ci][:, j * 128:(j + 1) * 128],
                                         h1[ci][:], start=(ci == 0),
                                         stop=(ci == ND - 1))
                    kt = sb.tile([128, T], BF16, tag="ktl", bufs=3, name="ktl")
                    nc.vector.tensor_copy(kt[:], mm[:])
                    half, hp_in = j // 4, j % 4
                    nc.sync.dma_start(
                        kv_in[half][hp_in * 128 * T:(hp_in + 1) * 128 * T]
                        .rearrange("(p n) -> p n", p=128), kt[:])

                # --- V projection; nh half -> AG half ---
                wv_t = []
                for ci in range(ND):
                    t = sb.tile([128, D], F32R, tag="w", bufs=9, name=f"wv{ci}")
                    nc.sync.dma_start(t[:], wv[l][ci * 128:(ci + 1) * 128, :])
                    wv_t.append(t)
                vaug = []
                for tt in range(NT):
                    va = sb.tile([128, VW], BF16, tag="kvg", bufs=16,
                                 name=f"va{tt}")
                    nc.sync.dma_start(
                        va[:, 0:VW].rearrange("p (h c) -> p h c", c=65)[:, :, 64:65],
                        onescol[:, :, None])
                    vaug.append(va)
                for nh in range(2):
                    for tt in range(NT):
                        mm = ps.tile([128, T], F32, tag="mm", bufs=2, name="vmm")
                        for ci in range(ND):
                            nc.tensor.matmul(
                                mm[:],
                                h1[ci][:, tt * 128:(tt + 1) * 128],
                                wv_t[ci][:, nh * 512:(nh + 1) * 512],
                                start=(ci == 0), stop=(ci == ND - 1))
                        nc.vector.tensor_copy(
                            vaug[tt][:, nh * 520:(nh + 1) * 520]
                            .rearrange("p (h c) -> p h c", c=65)[:, :, 0:64],
                            mm[:].rearrange("p (h c) -> p h c", c=64))
                        nc.sync.dma_start(
                            kv_in[nh][KTH_FLAT + tt * 128 * 520:
                                      KTH_FLAT + (tt + 1) * 128 * 520]
                            .rearrange("(p n) -> p n", p=128),
                            vaug[tt][:, nh * 520:(nh + 1) * 520])

                kv_out = []
                for h in range(2):
                    ko = dram.tile([4 * KVH_FLAT], BF16, tag=f"kvout{h}",
                                   bufs=2, name=f"kvout{h}")
                    nc.gpsimd.collective_compute(
                        "AllGather", ALU.bypass, replica_groups=groups,
                        ins=[kv_in[h].opt()], outs=[ko.opt()])
                    kv_out.append(ko)

                # --- Q projection ---
                wq_t = []
                for ci in range(ND):
                    t = sb.tile([128, D], F32R, tag="w", bufs=9, name=f"wq{ci}")
                    nc.sync.dma_start(t[:], wq[l][ci * 128:(ci + 1) * 128, :])
                    wq_t.append(t)
                qts = []
                for j in range(ND):
                    mm = ps.tile([128, T], F32, tag="mm", bufs=2, name="qmm")
                    for ci in range(ND):
                        nc.tensor.matmul(mm[:], wq_t[ci][:, j * 128:(j + 1) * 128],
                                         h1[ci][:], start=(ci == 0),
                                         stop=(ci == ND - 1))
                    qt = sb.tile([128, T], BF16, tag="qt", bufs=ND, name=f"qt{j}")
                    nc.vector.tensor_copy(qt[:], mm[:])
                    qts.append(qt)

                # --- attention (hp 0-3 from AG half 0, hp 4-7 from half 1) ---
                ctx_sb = []
                vfs = None
                for hp in range(HPAIRS):
                    half, hp_in = hp // 4, hp % 4
                    ko = kv_out[half]
                    ktf = sb.tile([128, S], BF16, tag="ktf", bufs=2,
                                  name=f"ktf{hp}")
                    for r in range(4):
                        off = r * KVH_FLAT + hp_in * 128 * T
                        nc.sync.dma_start(
                            ktf[:, r * T:(r + 1) * T],
                            ko[off:off + 128 * T]
                            .rearrange("(p n) -> p n", p=128))
                    if hp_in == 0:
                        # load this half's V tiles [128, 520] x 16
                        vfs = []
                        for kt_i in range(NK):
                            r, tt = kt_i // NT, kt_i % NT
                            off = r * KVH_FLAT + KTH_FLAT + tt * 128 * 520
                            vt = sb.tile([128, 520], BF16, tag="kvg", bufs=16,
                                         name=f"vf{half}_{kt_i}")
                            nc.sync.dma_start(
                                vt[:], ko[off:off + 128 * 520]
                                .rearrange("(p n) -> p n", p=128))
                            vfs.append(vt)
                    cs = sb.tile([128, T], F32R, tag="hx", bufs=ND,
                                 name=f"cs{hp}")
                    ctx_sb.append(cs)
                    ctxp = [ps.tile([65, T], F32, tag="ctxp", bufs=2,
                                    name=f"ctxp{hh}") for hh in range(2)]
                    for kt_i in range(NK):
                        sTs = []
                        for hh in range(2):
                            offp = hh * 64
                            sT = ps.tile([128, T], F32, tag="sT", bufs=2,
                                         name="sT")
                            nc.tensor.matmul(
                                sT[:],
                                ktf[offp:offp + 64,
                                    kt_i * 128:(kt_i + 1) * 128],
                                qts[hp][offp:offp + 64, :],
                                start=True, stop=True)
                            sTs.append(sT)
                        for hh in range(2):
                            h_loc = hp_in * 2 + hh   # head index within half
                            es = sb.tile([128, T], BF16, tag="work512", bufs=4,
                                         name="es")
                            nc.scalar.activation(es[:], sTs[hh][:], AF.Exp)
                            nc.vector.tensor_mul(es[:], es[:], mts[kt_i][:])
                            nc.tensor.matmul(
                                ctxp[hh][:],
                                vfs[kt_i][:, h_loc * 65:h_loc * 65 + 65],
                                es[:], start=(kt_i == 0),
                                stop=(kt_i == NK - 1))
                    for hh in range(2):
                        offp = hh * 64
                        rec = sb.tile([1, T], F32R, tag="lnsmall", bufs=3,
                                      name="rec")
                        with nc.allow_low_precision(reason="fp32r matmul feed"):
                            nc.vector.reciprocal(rec[:], ctxp[hh][64:65, :])
                        rbp = ps.tile([64, T], F32, tag="mm", bufs=2,
                                      name="rbp")
                        nc.tensor.matmul(rbp[:], ones1[0:1, 0:64], rec[:],
                                         start=True, stop=True)
                        nc.vector.tensor_copy(cs[offp:offp + 64, :],
                                              ctxp[hh][0:64, :])
                        nc.vector.tensor_mul(cs[offp:offp + 64, :],
                                             cs[offp:offp + 64, :], rbp[:])

                # --- Wo + residual ---
                wo_t = []
                for ci in range(ND):
                    t = sb.tile([128, D], F32R, tag="w", bufs=9, name=f"wo{ci}")
                    nc.sync.dma_start(t[:], wo[l][ci * 128:(ci + 1) * 128, :])
                    wo_t.append(t)
                for j in range(ND):
                    mm = ps.tile([128, T], F32, tag="mm", bufs=2, name="omm")
                    for ci in range(ND):
                        nc.tensor.matmul(mm[:], wo_t[ci][:, j * 128:(j + 1) * 128],
                                         ctx_sb[ci][:], start=(ci == 0),
                                         stop=(ci == ND - 1))
                    nc.vector.tensor_add(xts[j][:], xts[j][:], mm[:])

                h2 = layer_norm(xts, lg2, lb2, f"b{l}")

                # --- FFN: W1 + gelu for all 32 f-tiles ---
                gts = []
                for fi in range(NF):
                    slab = sb.tile([128, D], F32R, tag="w", bufs=9,
                                   name=f"w1s{fi}")
                    nc.sync.dma_start(slab[:], w1s[l, fi])
                    h3 = ps.tile([128, T], F32, tag="ctxp", bufs=2, name="h3")
                    for ci in range(ND):
                        nc.tensor.matmul(h3[:], slab[:, ci * 128:(ci + 1) * 128],
                                         h2[ci][:], start=(ci == 0),
                                         stop=(ci == ND - 1))
                    if fi % 2 == 0:
                        gt = sb.tile([128, 2 * T], F32R, tag="kvg", bufs=16,
                                     name=f"g{fi // 2}")
                        gts.append(gt)
                    nc.scalar.activation(
                        gts[fi // 2][:, (fi % 2) * T:(fi % 2 + 1) * T],
                        h3[:], AF.Gelu, bias=b1t[:, fi:fi + 1])

                # --- FFN: W2 single pass, 8 psum accumulators ---
                accs = []
                for j in range(ND):
                    tagj = ["mm", "mm", "sT", "sT", "ctxp", "ctxp", "stat",
                            "stat"][j]
                    accs.append(ps.tile([128, T], F32, tag=tagj, bufs=2,
                                        name=f"w2acc{j}"))
                for fi in range(NF):
                    slab = sb.tile([128, D], F32R, tag="w", bufs=9,
                                   name=f"w2s{fi}")
                    nc.sync.dma_start(slab[:], w2[l][fi * 128:(fi + 1) * 128, :])
                    for j in range(ND):
                        nc.tensor.matmul(
                            accs[j][:], slab[:, j * 128:(j + 1) * 128],
                            gts[fi // 2][:, (fi % 2) * T:(fi % 2 + 1) * T],
                            start=(fi == 0), stop=(fi == NF - 1))
                for j in range(ND):
                    nc.vector.scalar_tensor_tensor(
                        xts[j][:], accs[j][:], b2t[:, j:j + 1], xts[j][:],
                        ALU.add, ALU.add)

            # --- final LN ---
            lgf = sb.tile([128, ND], F32, tag="lnp", bufs=8, name="lgf")
            nc.sync.dma_start(lgf[:], lnfg.rearrange("(c p) -> p c", p=128))
            lbf = sb.tile([128, ND], F32, tag="lnp", bufs=8, name="lbf")
            nc.sync.dma_start(lbf[:], lnfb.rearrange("(c p) -> p c", p=128))
            hf = layer_norm(xts, lgf, lbf, "f")

            # --- LM head: vocab tiles ---
            for v in range(n_vtiles):
                slab = sb.tile([128, D], F32R, tag="w", bufs=9,
                               name=f"wouts{v}")
                nc.sync.dma_start(slab[:], woutr[v])
                mm = ps.tile([128, T], F32, tag="mm", bufs=2, name="lmm")
                for ci in range(ND):
                    nc.tensor.matmul(mm[:], slab[:, ci * 128:(ci + 1) * 128],
                                     hf[ci][:], start=(ci == 0),
                                     stop=(ci == ND - 1))
                ot = sb.tile([128, T], F32R, tag="work512", bufs=4, name="ot")
                nc.vector.tensor_copy(ot[:], mm[:])
                nc.sync.dma_start(logits_t[v * 128:(v + 1) * 128, :], ot[:])

    nc.compile()
    return nc


def get_program(n_layers=L, n_vtiles=NV):
    key = (n_layers, n_vtiles)
    if key not in _CACHE:
        _CACHE[key] = _build(n_layers, n_vtiles)
    return _CACHE[key]


def prep_inputs(tokens, tok_emb, pos_emb, Wq, Wk, Wv, Wo, ln1_g, ln1_b,
                ln2_g, ln2_b, W1, b1, W2, b2, lnf_g, lnf_b, Wout):
    import ml_dtypes
    tokens = np.asarray(tokens)
    f = lambda a: np.ascontiguousarray(np.asarray(a, dtype=np.float32))
    tok_emb, pos_emb = f(tok_emb), f(pos_emb)
    Wq, Wk, Wv, Wo = f(Wq), f(Wk), f(Wv), f(Wo)
    W1, W2, Wout = f(W1), f(W2), f(Wout)
    ln1_g, ln1_b, ln2_g, ln2_b = f(ln1_g), f(ln1_b), f(ln2_g), f(ln2_b)
    b1a, b2a, lnf_g, lnf_b = f(b1), f(b2), f(lnf_g), f(lnf_b)

    wq_s = np.ascontiguousarray(Wq / np.sqrt(DK))   # fold 1/sqrt(dk) into Q
    w1s = np.ascontiguousarray(
        W1.reshape(L, ND, 128, NF, 128).transpose(0, 3, 2, 1, 4)
        .reshape(L, NF, 128, D))
    woutr = np.ascontiguousarray(
        Wout.reshape(ND, 128, NV, 128).transpose(2, 1, 0, 3)
        .reshape(NV, 128, D))
    ones_in = np.ones(128, np.float32)
    onescol = np.ones((128, H), ml_dtypes.bfloat16)

    shared = dict(wq=wq_s, wk=Wk, wv=Wv, wo=Wo, w1s=w1s, w2=W2, woutr=woutr,
                  ln1g=ln1_g, ln1b=ln1_b, ln2g=ln2_g, ln2b=ln2_b,
                  b1=b1a, b2=b2a, lnfg=lnf_g, lnfb=lnf_b,
                  ones_in=ones_in, onescol=onescol)

    in_maps = []
    for c in range(N_CORES):
        g, r = c // 4, c % 4
        toks = tokens[g, r * T:(r + 1) * T]
        x0 = tok_emb[toks] + pos_emb[r * T:(r + 1) * T]
        x0t = np.ascontiguousarray(x0.T)
        k_idx = np.arange(S)[:, None]
        q_idx = r * T + np.arange(T)[None, :]
        maskt = (k_idx <= q_idx).astype(ml_dtypes.bfloat16)
        m = dict(shared)
        m["x0t"] = x0t
        m["maskt"] = maskt
        in_maps.append(m)
    return in_maps


def kernel(**inputs):
    nc = get_program()
    in_maps = prep_inputs(**inputs)
    res = run_bass_kernel_spmd(nc, in_maps, list(range(N_CORES)))
    out = np.empty((B, S, V), np.float32)
    for c in range(N_CORES):
        g, r = c // 4, c % 4
        out[g, r * T:(r + 1) * T, :] = res.results[c]["logits_t"].T
    return out



# revision 10
# speedup vs baseline: 1.4293x; 1.4293x over previous
"""MiniGPT forward pass on 8 Trainium2 NeuronCores.

Sharding: sequence-parallel with block-interleaved token assignment.
Core c handles batch g = c//4 and query tiles {r, r+4, r+8, r+12}
(r = c%4, tiles of 128 tokens).  With this mapping, query tile j
(j = 0..3) of EVERY core only attends to key tiles kt < 4*(j+1), so a
single SPMD program skips the strictly-upper-triangular 37.5% of
attention compute uniformly; the per-core causal boundary inside the
diagonal block is handled by a per-core mask input.

K/V are exchanged with two 4-core AllGathers per layer (split by head
halves; AG0 is kicked as soon as half-0 K/V are projected so it
overlaps the rest of the projections).

All matmuls run in bf16 (weights converted host-side) with fp32 PSUM
accumulation; fp32r is only used for tiny [1,x] broadcast operands.
The residual stream stays fp32.  Softmax skips max-subtraction
(pre-softmax scores are O(1)); masked positions are zeroed by
multiplying exp(s) with a 0/1 mask on the diagonal 128-col block only;
the denominator comes from a ones-column appended to V inside the same
accumulation matmul.  Logits are emitted in bf16 and upcast on host.
"""
import sys
sys.path.insert(0, '/opt/trn_rl_repo')
import numpy as np
import concourse.bass as bass
import concourse.bacc as bacc
import concourse.tile as tile
import concourse.mybir as mybir
from concourse.bass_utils import run_bass_kernel_spmd

F32 = mybir.dt.float32
F32R = mybir.dt.float32r
BF16 = mybir.dt.bfloat16
AF = mybir.ActivationFunctionType
ALU = mybir.AluOpType

# model dims
B, S, D, H, DK, F, V = 2, 2048, 1024, 16, 64, 4096, 32000
L = 6
EPS = 1e-5
T = 512           # tokens per core
NT = T // 128     # 4 query tiles per core
ND = D // 128     # 8 d tiles
NF = F // 128     # 32 f tiles
NK = S // 128     # 16 key tiles
NV = V // 128     # 250 vocab tiles
HPAIRS = H // 2   # 8 head-pair tiles
N_CORES = 8

# one AG half: K rows [h*512:(h+1)*512] (4 hp tiles) + V cols [h*520:(h+1)*520]
KTH_FLAT = 4 * 128 * T          # 262144
VH_FLAT = T * 520               # 266240
KVH_FLAT = KTH_FLAT + VH_FLAT   # 528384
VW = H * 65                     # 1040

_CACHE = {}


def _build(n_layers=L, n_vtiles=NV):
    nc = bacc.Bacc("TRN2", target_bir_lowering=False, debug=False,
                   num_devices=N_CORES)

    def din(name, shape, dt=BF16):
        return nc.dram_tensor(name, shape, dt, kind="ExternalInput").ap()

    x0t = din("x0t", [D, T], F32R)
    mask16 = din("mask16", [NK, 128, 256], BF16)
    wq = din("wq", [L, D, D])
    wk = din("wk", [L, D, D])
    wv = din("wv", [L, D, D])
    wo = din("wo", [L, D, D])
    w1s = din("w1s", [L, NF, 128, D])
    w2 = din("w2", [L, F, D])
    woutr = din("woutr", [NV, 128, D])
    ln1g = din("ln1g", [L, D], F32)
    ln1b = din("ln1b", [L, D], F32)
    ln2g = din("ln2g", [L, D], F32)
    ln2b = din("ln2b", [L, D], F32)
    b1 = din("b1", [L, F], F32)
    b2 = din("b2", [L, D], F32)
    lnfg = din("lnfg", [D], F32)
    lnfb = din("lnfb", [D], F32)
    onesd_f = din("onesd_f", [128], F32R)   # value 1/D
    onesd_b = din("onesd_b", [128], BF16)   # value 1/D
    ones_f = din("ones_f", [128], F32R)     # value 1
    ones_b = din("ones_b", [128], BF16)     # value 1
    onescol = din("onescol", [128, H], BF16)

    logits_t = nc.dram_tensor("logits_t", [n_vtiles * 128, T], BF16,
                              kind="ExternalOutput").ap()

    groups = [[0, 1, 2, 3], [4, 5, 6, 7]]
    # storage slot (in ktf columns / vfs list) for global key tile kt
    slot_of_kt = [(kt % 4) * 4 + kt // 4 for kt in range(NK)]

    with tile.TileContext(nc) as tc:
        with tc.tile_pool(name="sb", bufs=1) as sb, \
             tc.tile_pool(name="ps", bufs=1, space="PSUM") as ps, \
             tc.tile_pool(name="dram", bufs=1, space="DRAM") as dram:

            onesd = sb.tile([128, 1], F32R, tag="onesd")
            onesdb = sb.tile([128, 1], BF16, tag="onesdb")
            ones1 = sb.tile([1, 128], F32R, tag="ones1")
            ones1b = sb.tile([1, 128], BF16, tag="ones1b")
            nc.sync.dma_start(onesd[:], onesd_f[:, None])
            nc.sync.dma_start(onesdb[:], onesd_b[:, None])
            nc.sync.dma_start(ones1[:], ones_f[None, :])
            nc.sync.dma_start(ones1b[:], ones_b[None, :])

            # persistent residual stream xT: 8 tiles [128, 512] fp32
            xts = []
            for j in range(ND):
                t = sb.tile([128, T], F32R, tag="xt", bufs=ND, name=f"xt{j}")
                nc.sync.dma_start(t[:], x0t[j * 128:(j + 1) * 128, :])
                xts.append(t)

            # diagonal-block masks (2-head duplicated), resident whole kernel
            mts = []
            for kt in range(NK):
                t = sb.tile([128, 256], BF16, tag="mask", bufs=NK,
                            name=f"mask{kt}")
                nc.sync.dma_start(t[:], mask16[kt])
                mts.append(t)

            def layer_norm(x_tiles, gc, bc, sfx):
                # stats: cols 0:512 = mean (ones scaled by 1/D), 512: = E[x^2]
                stats = ps.tile([1, 1024], F32, tag="big", bufs=2,
                                name=f"st{sfx}")
                for j in range(ND):
                    sq = sb.tile([128, T], BF16, tag="work512", bufs=4,
                                 name=f"sq{sfx}")
                    nc.vector.tensor_mul(sq[:], x_tiles[j][:], x_tiles[j][:])
                    nc.tensor.matmul(stats[0:1, 0:T], onesd[:], x_tiles[j][:],
                                     start=(j == 0), stop=(j == ND - 1))
                    nc.tensor.matmul(stats[0:1, T:2 * T], onesdb[:], sq[:],
                                     start=(j == 0), stop=(j == ND - 1))
                meansb = sb.tile([1, T], BF16, tag="lnsmall", bufs=4,
                                 name=f"mn{sfx}")
                nc.vector.tensor_copy(meansb[:], stats[0:1, 0:T])
                mean2 = sb.tile([1, T], F32, tag="lnsmall", bufs=4,
                                name=f"m2{sfx}")
                nc.vector.tensor_mul(mean2[:], meansb[:], meansb[:])
                var = sb.tile([1, T], F32, tag="lnsmall", bufs=4,
                              name=f"var{sfx}")
                nc.vector.scalar_tensor_tensor(var[:], stats[0:1, T:2 * T],
                                               EPS, mean2[:], ALU.add,
                                               ALU.subtract)
                sd = sb.tile([1, T], F32, tag="lnsmall", bufs=4,
                             name=f"sd{sfx}")
                nc.scalar.activation(sd[:], var[:], AF.Sqrt)
                rstd = sb.tile([1, T], F32, tag="lnsmall", bufs=4,
                               name=f"rstd{sfx}")
                nc.vector.reciprocal_approx_fast(rstd[:], sd[:])
                rstdb = sb.tile([1, T], BF16, tag="lnsmall", bufs=4,
                                name=f"rsb{sfx}")
                nc.vector.tensor_copy(rstdb[:], rstd[:])
                # mbrb: cols 0:512 = broadcast mean, 512: = broadcast rstd
                mbrb = ps.tile([128, 1024], F32, tag="big", bufs=2,
                               name=f"mbrb{sfx}")
                nc.tensor.matmul(mbrb[:, 0:T], ones1b[:], meansb[:],
                                 start=True, stop=True)
                nc.tensor.matmul(mbrb[:, T:2 * T], ones1b[:], rstdb[:],
                                 start=True, stop=True)
                h_tiles = []
                for j in range(ND):
                    t1 = sb.tile([128, T], F32, tag="workf", bufs=4,
                                 name=f"t1{sfx}")
                    nc.vector.tensor_sub(t1[:], x_tiles[j][:], mbrb[:, 0:T])
                    nc.vector.tensor_mul(t1[:], t1[:], mbrb[:, T:2 * T])
                    ht = sb.tile([128, T], BF16, tag="hx", bufs=ND,
                                 name=f"h{sfx}_{j}")
                    nc.scalar.activation(ht[:], t1[:], AF.Identity,
                                         bias=bc[:, j:j + 1],
                                         scale=gc[:, j:j + 1])
                    h_tiles.append(ht)
                return h_tiles

            for l in range(n_layers):
                # --- per-layer params ---
                lg1 = sb.tile([128, ND], F32, tag="lnp", bufs=8, name="lg1")
                nc.sync.dma_start(lg1[:], ln1g[l].rearrange("(c p) -> p c", p=128))
                lb1 = sb.tile([128, ND], F32, tag="lnp", bufs=8, name="lb1")
                nc.sync.dma_start(lb1[:], ln1b[l].rearrange("(c p) -> p c", p=128))
                lg2 = sb.tile([128, ND], F32, tag="lnp", bufs=8, name="lg2")
                nc.sync.dma_start(lg2[:], ln2g[l].rearrange("(c p) -> p c", p=128))
                lb2 = sb.tile([128, ND], F32, tag="lnp", bufs=8, name="lb2")
                nc.sync.dma_start(lb2[:], ln2b[l].rearrange("(c p) -> p c", p=128))
                b1t = sb.tile([128, NF], F32, tag="b1t", bufs=2, name="b1t")
                nc.sync.dma_start(b1t[:], b1[l].rearrange("(c p) -> p c", p=128))
                b2t = sb.tile([128, ND], F32, tag="lnp", bufs=8, name="b2t")
                nc.sync.dma_start(b2t[:], b2[l].rearrange("(c p) -> p c", p=128))

                h1 = layer_norm(xts, lg1, lb1, f"a{l}")

                kv_in = [dram.tile([KVH_FLAT], BF16, tag=f"kvin{h}", bufs=2,
                                   name=f"kvin{h}") for h in range(2)]
                kv_out = []

                wk_t = []
                for ci in range(ND):
                    t = sb.tile([128, D], BF16, tag="w", bufs=18, name=f"wk{ci}")
                    nc.sync.dma_start(t[:], wk[l][ci * 128:(ci + 1) * 128, :])
                    wk_t.append(t)
                wv_t = []
                for ci in range(ND):
                    t = sb.tile([128, D], BF16, tag="w", bufs=18, name=f"wv{ci}")
                    nc.sync.dma_start(t[:], wv[l][ci * 128:(ci + 1) * 128, :])
                    wv_t.append(t)
                vaug = []
                for tt in range(NT):
                    va = sb.tile([128, VW], BF16, tag="kvg", bufs=16,
                                 name=f"va{tt}")
                    nc.sync.dma_start(
                        va[:, 0:VW].rearrange("p (h c) -> p h c", c=65)[:, :, 64:65],
                        onescol[:, :, None])
                    vaug.append(va)

                # --- K + V projections per half; AG as soon as a half is done
                for half in range(2):
                    # K tiles j = 4*half .. 4*half+3
                    for jp in range(2):
                        mm = ps.tile([128, 1024], F32, tag="big", bufs=2,
                                     name="kmm")
                        for sub in range(2):
                            j = half * 4 + jp * 2 + sub
                            for ci in range(ND):
                                nc.tensor.matmul(
                                    mm[:, sub * T:(sub + 1) * T],
                                    wk_t[ci][:, j * 128:(j + 1) * 128],
                                    h1[ci][:], start=(ci == 0),
                                    stop=(ci == ND - 1))
                        for sub in range(2):
                            hp_in = jp * 2 + sub
                            kt_sb = sb.tile([128, T], BF16, tag="ktl", bufs=4,
                                            name="ktl")
                            nc.vector.tensor_copy(kt_sb[:],
                                                  mm[:, sub * T:(sub + 1) * T])
                            nc.sync.dma_start(
                                kv_in[half][hp_in * 128 * T:(hp_in + 1) * 128 * T]
                                .rearrange("(p n) -> p n", p=128), kt_sb[:])
                    # V for this half (vcols half*512 .. +512)
                    for tp in range(2):
                        mm = ps.tile([128, 1024], F32, tag="big", bufs=2,
                                     name="vmm")
                        for sub in range(2):
                            tt = tp * 2 + sub
                            for ci in range(ND):
                                nc.tensor.matmul(
                                    mm[:, sub * T:(sub + 1) * T],
                                    h1[ci][:, tt * 128:(tt + 1) * 128],
                                    wv_t[ci][:, half * T:(half + 1) * T],
                                    start=(ci == 0), stop=(ci == ND - 1))
                        for sub in range(2):
                            tt = tp * 2 + sub
                            nc.vector.tensor_copy(
                                vaug[tt][:, half * 520:(half + 1) * 520]
                                .rearrange("p (h c) -> p h c", c=65)[:, :, 0:64],
                                mm[:, sub * T:(sub + 1) * T]
                                .rearrange("p (h c) -> p h c", c=64))
                            nc.sync.dma_start(
                                kv_in[half][KTH_FLAT + tt * 128 * 520:
                                            KTH_FLAT + (tt + 1) * 128 * 520]
                                .rearrange("(p n) -> p n", p=128),
                                vaug[tt][:, half * 520:(half + 1) * 520])

                for half in range(2):
                    ko = dram.tile([4 * KVH_FLAT], BF16, tag=f"kvout{half}",
                                   bufs=2, name=f"kvout{half}")
                    nc.gpsimd.collective_compute(
                        "AllGather", ALU.bypass, replica_groups=groups,
                        ins=[kv_in[half].opt()], outs=[ko.opt()])
                    kv_out.append(ko)

                # --- Q projection ---
                wq_t = []
                for ci in range(ND):
                    t = sb.tile([128, D], BF16, tag="w", bufs=18, name=f"wq{ci}")
                    nc.sync.dma_start(t[:], wq[l][ci * 128:(ci + 1) * 128, :])
                    wq_t.append(t)
                qts = []
                for jp in range(4):
                    mm = ps.tile([128, 1024], F32, tag="big", bufs=2,
                                 name="qmm")
                    for sub in range(2):
                        j = jp * 2 + sub
                        for ci in range(ND):
                            nc.tensor.matmul(
                                mm[:, sub * T:(sub + 1) * T],
                                wq_t[ci][:, j * 128:(j + 1) * 128],
                                h1[ci][:], start=(ci == 0),
                                stop=(ci == ND - 1))
                    for sub in range(2):
                        qt = sb.tile([128, T], BF16, tag="qt", bufs=ND,
                                     name=f"qt{jp * 2 + sub}")
                        nc.vector.tensor_copy(qt[:], mm[:, sub * T:(sub + 1) * T])
                        qts.append(qt)

                # --- attention (hp 0-3 from AG half 0, hp 4-7 from half 1) ---
                ctx_sb = []
                vfs = None
                for hp in range(HPAIRS):
                    half, hp_in = hp // 4, hp % 4
                    ko = kv_out[half]
                    ktf = sb.tile([128, S], BF16, tag="ktf", bufs=2,
                                  name=f"ktf{hp}")
                    for r in range(4):
                        off = r * KVH_FLAT + hp_in * 128 * T
                        nc.sync.dma_start(
                            ktf[:, r * T:(r + 1) * T],
                            ko[off:off + 128 * T]
                            .rearrange("(p n) -> p n", p=128))
                    if hp_in == 0:
                        vfs = []
                        for i in range(NK):
                            r, tt = i // NT, i % NT
                            off = r * KVH_FLAT + KTH_FLAT + tt * 128 * 520
                            vt = sb.tile([128, 520], BF16, tag="kvg", bufs=16,
                                         name=f"vf{half}_{i}")
                            nc.sync.dma_start(
                                vt[:], ko[off:off + 128 * 520]
                                .rearrange("(p n) -> p n", p=128))
                            vfs.append(vt)
                    ctxp = [ps.tile([65, T], F32, tag="m1", bufs=2,
                                    name=f"ctxp{hh}") for hh in range(2)]
                    for kt in range(NK):
                        s = slot_of_kt[kt]
                        jmin = kt // 4
                        off = jmin * 128
                        es_ps = ps.tile([128, 1024], F32, tag="big", bufs=2,
                                        name="es_ps")
                        for hh in range(2):
                            op = hh * 64
                            nc.tensor.matmul(
                                es_ps[:, hh * T + off:(hh + 1) * T],
                                ktf[op:op + 64, s * 128:(s + 1) * 128],
                                qts[hp][op:op + 64, off:T],
                                start=True, stop=True)
                        es_sb = sb.tile([128, 1024], BF16, tag="es", bufs=3,
                                        name="es_sb")
                        nc.scalar.activation(
                            es_sb.rearrange("p (h n) -> p h n", h=2)[:, :, off:T],
                            es_ps.rearrange("p (h n) -> p h n", h=2)[:, :, off:T],
                            AF.Exp)
                        # mask only the diagonal 128-col block (both heads)
                        nc.vector.tensor_mul(
                            es_sb.rearrange("p (h n) -> p h n", h=2)
                            [:, :, off:off + 128],
                            es_sb.rearrange("p (h n) -> p h n", h=2)
                            [:, :, off:off + 128],
                            mts[kt].rearrange("p (h n) -> p h n", h=2))
                        for hh in range(2):
                            h_loc = hp_in * 2 + hh
                            nc.tensor.matmul(
                                ctxp[hh][:, off:T],
                                vfs[s][:, h_loc * 65:h_loc * 65 + 65],
                                es_sb[:, hh * T + off:(hh + 1) * T],
                                start=(kt == 0), stop=(kt == NK - 1))
                    # finalize: denominators for both heads in one go
                    den = sb.tile([1, 1024], F32, tag="lnsmall", bufs=4,
                                  name="den")
                    nc.vector.tensor_copy(den[0:1, 0:T], ctxp[0][64:65, :])
                    nc.vector.tensor_copy(den[0:1, T:2 * T], ctxp[1][64:65, :])
                    rec = sb.tile([1, 1024], F32, tag="lnsmall", bufs=4,
                                  name="rec")
                    nc.vector.reciprocal_approx_fast(rec[:], den[:])
                    recb = sb.tile([1, 1024], BF16, tag="lnsmall", bufs=4,
                                   name="recb")
                    nc.vector.tensor_copy(recb[:], rec[:])
                    cs = sb.tile([128, T], BF16, tag="cs", bufs=ND,
                                 name=f"cs{hp}")
                    ctx_sb.append(cs)
                    for hh in range(2):
                        rbp = ps.tile([64, T], F32, tag="m2", bufs=2,
                                      name="rbp")
                        nc.tensor.matmul(
                            rbp[:], ones1b[0:1, 0:64],
                            recb[0:1, hh * T:(hh + 1) * T],
                            start=True, stop=True)
                        nc.vector.tensor_copy(cs[hh * 64:(hh + 1) * 64, :],
                                              ctxp[hh][0:64, :])
                        nc.vector.tensor_mul(cs[hh * 64:(hh + 1) * 64, :],
                                             cs[hh * 64:(hh + 1) * 64, :],
                                             rbp[:])

                # --- Wo + residual ---
                wo_t = []
                for ci in range(ND):
                    t = sb.tile([128, D], BF16, tag="w", bufs=18, name=f"wo{ci}")
                    nc.sync.dma_start(t[:], wo[l][ci * 128:(ci + 1) * 128, :])
                    wo_t.append(t)
                for jp in range(4):
                    mm = ps.tile([128, 1024], F32, tag="big", bufs=2,
                                 name="omm")
                    for sub in range(2):
                        j = jp * 2 + sub
                        for ci in range(ND):
                            nc.tensor.matmul(
                                mm[:, sub * T:(sub + 1) * T],
                                wo_t[ci][:, j * 128:(j + 1) * 128],
                                ctx_sb[ci][:], start=(ci == 0),
                                stop=(ci == ND - 1))
                    for sub in range(2):
                        j = jp * 2 + sub
                        nc.vector.tensor_add(xts[j][:], xts[j][:],
                                             mm[:, sub * T:(sub + 1) * T])

                h2 = layer_norm(xts, lg2, lb2, f"b{l}")

                # --- FFN: W1 + gelu; h3 tiles in m1/m2 so the W2 wave-A
                # accumulators (big) never contend with them ---
                gts = []
                for fi in range(NF):
                    slab = sb.tile([128, D], BF16, tag="w", bufs=18,
                                   name=f"w1s{fi}")
                    nc.sync.dma_start(slab[:], w1s[l, fi])
                    h3 = ps.tile([128, T], F32, tag=("m1" if fi % 2 == 0
                                                     else "m2"), bufs=2,
                                 name="h3")
                    for ci in range(ND):
                        nc.tensor.matmul(h3[:], slab[:, ci * 128:(ci + 1) * 128],
                                         h2[ci][:], start=(ci == 0),
                                         stop=(ci == ND - 1))
                    if fi % 2 == 0:
                        gt = sb.tile([128, 2 * T], BF16, tag="kvg", bufs=16,
                                     name=f"g{fi // 2}")
                        gts.append(gt)
                    nc.scalar.activation(
                        gts[fi // 2][:, (fi % 2) * T:(fi % 2 + 1) * T],
                        h3[:], AF.Gelu, bias=b1t[:, fi:fi + 1])

                # --- FFN: W2 in two waves (A: j 0-3 in big pairs, B: j 4-7
                # in m1/m2 singles after W1's h3 tiles are all released) ---
                accA = [ps.tile([128, 1024], F32, tag="big", bufs=2,
                                name=f"w2a{p}") for p in range(2)]
                for fi in range(NF):
                    slab = sb.tile([128, D], BF16, tag="w", bufs=18,
                                   name=f"w2sa{fi}")
                    nc.sync.dma_start(slab[:], w2[l][fi * 128:(fi + 1) * 128, :])
                    for j in range(4):
                        nc.tensor.matmul(
                            accA[j // 2][:, (j % 2) * T:(j % 2 + 1) * T],
                            slab[:, j * 128:(j + 1) * 128],
                            gts[fi // 2][:, (fi % 2) * T:(fi % 2 + 1) * T],
                            start=(fi == 0), stop=(fi == NF - 1))
                for j in range(4):
                    nc.vector.scalar_tensor_tensor(
                        xts[j][:], accA[j // 2][:, (j % 2) * T:(j % 2 + 1) * T],
                        b2t[:, j:j + 1], xts[j][:], ALU.add, ALU.add)
                accB = [ps.tile([128, T], F32, tag=("m1" if p < 2 else "m2"),
                                bufs=2, name=f"w2b{p}") for p in range(4)]
                for fi in range(NF):
                    slab = sb.tile([128, D], BF16, tag="w", bufs=18,
                                   name=f"w2sb{fi}")
                    nc.sync.dma_start(slab[:], w2[l][fi * 128:(fi + 1) * 128, :])
                    for p in range(4):
                        j = 4 + p
                        nc.tensor.matmul(
                            accB[p][:], slab[:, j * 128:(j + 1) * 128],
                            gts[fi // 2][:, (fi % 2) * T:(fi % 2 + 1) * T],
                            start=(fi == 0), stop=(fi == NF - 1))
                for p in range(4):
                    j = 4 + p
                    nc.vector.scalar_tensor_tensor(
                        xts[j][:], accB[p][:], b2t[:, j:j + 1], xts[j][:],
                        ALU.add, ALU.add)

            # --- final LN ---
            lgf = sb.tile([128, ND], F32, tag="lnp", bufs=8, name="lgf")
            nc.sync.dma_start(lgf[:], lnfg.rearrange("(c p) -> p c", p=128))
            lbf = sb.tile([128, ND], F32, tag="lnp", bufs=8, name="lbf")
            nc.sync.dma_start(lbf[:], lnfb.rearrange("(c p) -> p c", p=128))
            hf = layer_norm(xts, lgf, lbf, "f")

            # --- LM head: vocab tile pairs; bf16 logits out ---
            for vp in range(n_vtiles // 2):
                slabs = []
                for sub in range(2):
                    sl = sb.tile([128, D], BF16, tag="w", bufs=18,
                                 name=f"wouts{vp}_{sub}")
                    nc.sync.dma_start(sl[:], woutr[vp * 2 + sub])
                    slabs.append(sl)
                mm = ps.tile([128, 1024], F32, tag="big", bufs=2, name="lmm")
                for sub in range(2):
                    for ci in range(ND):
                        nc.tensor.matmul(
                            mm[:, sub * T:(sub + 1) * T],
                            slabs[sub][:, ci * 128:(ci + 1) * 128],
                            hf[ci][:], start=(ci == 0), stop=(ci == ND - 1))
                for sub in range(2):
                    v = vp * 2 + sub
                    ot = sb.tile([128, T], BF16, tag="work512", bufs=4,
                                 name="ot")
                    if sub == 0:
                        nc.vector.tensor_copy(ot[:], mm[:, 0:T])
                    else:
                        nc.scalar.copy(ot[:], mm[:, T:2 * T])
                    nc.sync.dma_start(logits_t[v * 128:(v + 1) * 128, :], ot[:])

    nc.compile()
    return nc


def get_program(n_layers=L, n_vtiles=NV):
    key = (n_layers, n_vtiles)
    if key not in _CACHE:
        _CACHE[key] = _build(n_layers, n_vtiles)
    return _CACHE[key]


def prep_inputs(tokens, tok_emb, pos_emb, Wq, Wk, Wv, Wo, ln1_g, ln1_b,
                ln2_g, ln2_b, W1, b1, W2, b2, lnf_g, lnf_b, Wout):
    import ml_dtypes
    BF = ml_dtypes.bfloat16
    tokens = np.asarray(tokens)
    f = lambda a: np.asarray(a, dtype=np.float32)
    h = lambda a: np.ascontiguousarray(np.asarray(a, dtype=np.float32)
                                       .astype(BF))
    tok_emb, pos_emb = f(tok_emb), f(pos_emb)
    wq_s = h(np.asarray(Wq, np.float32) / np.sqrt(DK))
    wk_b, wv_b, wo_b = h(Wk), h(Wv), h(Wo)
    w1s = h(np.asarray(W1, np.float32)
            .reshape(L, ND, 128, NF, 128).transpose(0, 3, 2, 1, 4)
            .reshape(L, NF, 128, D))
    w2_b = h(W2)
    woutr = h(np.asarray(Wout, np.float32)
              .reshape(ND, 128, NV, 128).transpose(2, 1, 0, 3)
              .reshape(NV, 128, D))
    ln1_g, ln1_b, ln2_g, ln2_b = f(ln1_g), f(ln1_b), f(ln2_g), f(ln2_b)
    b1a, b2a, lnf_g, lnf_b = f(b1), f(b2), f(lnf_g), f(lnf_b)

    shared = dict(wq=wq_s, wk=wk_b, wv=wv_b, wo=wo_b, w1s=w1s, w2=w2_b,
                  woutr=woutr, ln1g=ln1_g, ln1b=ln1_b, ln2g=ln2_g,
                  ln2b=ln2_b, b1=b1a, b2=b2a, lnfg=lnf_g, lnfb=lnf_b,
                  onesd_f=np.full(128, 1.0 / D, np.float32),
                  onesd_b=np.full(128, 1.0 / D, BF),
                  ones_f=np.ones(128, np.float32),
                  ones_b=np.ones(128, BF),
                  onescol=np.ones((128, H), BF))

    in_maps = []
    for c in range(N_CORES):
        g, r = c // 4, c % 4
        tiles = [r + 4 * a for a in range(NT)]
        tok_sel = np.concatenate(
            [tokens[g, t * 128:(t + 1) * 128] for t in tiles])
        pos_sel = np.concatenate(
            [pos_emb[t * 128:(t + 1) * 128] for t in tiles], axis=0)
        x0 = tok_emb[tok_sel] + pos_sel
        x0t = np.ascontiguousarray(x0.T)
        # diagonal-block masks: for key tile kt, query tile j0 = kt//4
        # (this core's queries in tile j0 are global tile 4*j0 + r)
        mask16 = np.empty((NK, 128, 256), BF)
        k_loc = np.arange(128)[:, None]
        q_loc = np.arange(128)[None, :]
        for kt in range(NK):
            j0 = kt // 4
            m = ((kt * 128 + k_loc) <= ((4 * j0 + r) * 128 + q_loc))
            mask16[kt] = np.tile(m.astype(BF), (1, 2))
        m = dict(shared)
        m["x0t"] = x0t
        m["mask16"] = mask16
        in_maps.append(m)
    return in_maps


def unshard_output(results):
    out = np.empty((B, S, V), np.float32)
    for c in range(N_CORES):
        g, r = c // 4, c % 4
        lt = np.asarray(results[c]["logits_t"]).astype(np.float32)
        for a in range(NT):
            t = 4 * a + r
            out[g, t * 128:(t + 1) * 128, :] = lt[:, a * 128:(a + 1) * 128].T
    return out


def kernel(**inputs):
    nc = get_program()
    in_maps = prep_inputs(**inputs)
    res = run_bass_kernel_spmd(nc, in_maps, list(range(N_CORES)))
    return unshard_output(res.results)
